# revision 67
# baseline (speedup 1.0000x reference)
"""Encoder-decoder GQA attention block (B=4, L=S=1024, H=2048, 32 Q heads,
8 KV heads, head_dim 64) + output projection + residual + layernorm, on 8
Trainium2 NeuronCores.

Sharding: rows. Core c handles batch c//2, L-half c%2 (512 query rows).
K/V projections are computed per-batch (duplicated on the 2 cores sharing a
batch), attention over all 32 heads for the core's rows, output projection,
residual + LN. No collectives.

v3 (on top of the fp8/DoubleRow v2 scheme):
- ACT exp stream (128 x [128,2,512] ~1.04us each = 133us) is the pole; the
  schedule minimizes head (first exp ~11us: wk/wq are repacked m-block-outer
  so the critical first slices are small DMAs) and tail.
- scores matmuls drop DoubleRow (K=64 real rows; the pad contributed zeros
  anyway): no kT/qT zero-pad memsets at all, PE engine has slack.
- O-proj is two-stage: chunks 0-5 (heads 0-23) accumulate into a transient
  PSUM tile late in phase B (in the 2 banks freed by finishing all q_proj by
  hh=14) and are staged to SBUF bf16; phase C runs chunks 6-7 + staged-add +
  residual per (tt,nb) group, then LN.
- LN rstd = exp(-0.5*ln(var+eps)): every activation (exp/ln/identity) lives
  in act table set 6, loaded once explicitly at t=0 and never switched --
  no 1.28us table load on the tail critical path.
- All input DMAs ride the in-order SP queue sorted by first-need time (the
  DMA-engine pool is exclusive, so transfer order == issue order); output
  stores too. Nothing on the ACT queue.
- Phase-C normalize runs entirely on ACT (idle once exps end); DVE keeps
  the 16-unit bn_stats stream, which is the tail's pole after the last
  head's softmax-normalize chain.

Scale bookkeeping (unchanged from v2): weights x32 on host; q,k carry x32
each -> scores PSUM x1024 -> exp scale 2^-13, bias -2; V ones column 1.0;
ctx = 32*ctx_true; O-proj PSUM = 1024*(ctx@Wo); xres pre-scaled x1024;
layernorm is scale-invariant.
"""

from contextlib import ExitStack

import numpy as np
import ml_dtypes

import concourse.bass as bass  # noqa: F401  (bass.AP used via handles)
import concourse.mybir as mybir
import concourse.tile as tile
from concourse import bacc
from concourse.bass_utils import run_bass_kernel_spmd

BF16 = ml_dtypes.bfloat16
FP8 = ml_dtypes.float8_e4m3fn

H = 2048
NH = 32
KVH = 8
G = 4           # query-head groups per kv head
HD = 64
B, L, S = 4, 1024, 1024
TOK = 512       # decoder rows per core
KC = 8          # contraction chunk-pairs (8 x (2x128) = 2048)
SC = S // 128   # 8 s chunks
EPS = 1e-6
WS = 32.0       # host-side fp8 weight scale

FP = mybir.dt.float32
BF = mybir.dt.bfloat16
F8 = mybir.dt.float8e4
DR = mybir.MatmulPerfMode.DoubleRow

_CACHE: dict = {}


def _build(use_mask: bool, triv_ln: bool, triv_bias: bool):
    nc = bacc.Bacc("TRN2", target_bir_lowering=False)

    xT = nc.dram_tensor("xT", [128, KC, 2, TOK], F8, kind="ExternalInput")
    xres = nc.dram_tensor("xres", [TOK, H], BF, kind="ExternalInput")
    eT = nc.dram_tensor("eT", [128, KC, 2, S], F8, kind="ExternalInput")
    # m-block-outer packing: [:, m] is one contiguous 128-col weight block
    wq = nc.dram_tensor("wq", [128, 16, KC, 2, 128], F8, kind="ExternalInput")
    wk = nc.dram_tensor("wk", [128, 4, KC, 2, 128], F8, kind="ExternalInput")
    wv = nc.dram_tensor("wv", [128, KC, 2, 512], F8, kind="ExternalInput")
    wo = nc.dram_tensor("wo", [128, KC, 2, H], F8, kind="ExternalInput")
    if not triv_bias:
        bias_all = nc.dram_tensor("bias_all", [128, 532], FP,
                                  kind="ExternalInput")
    ident = nc.dram_tensor("ident", [128, 128], BF, kind="ExternalInput")
    if not triv_ln:
        gamr = nc.dram_tensor("gamr", [128, H], BF, kind="ExternalInput")
        betr = nc.dram_tensor("betr", [128, H], BF, kind="ExternalInput")
    if use_mask:
        maskT = nc.dram_tensor("maskT", [S, TOK], BF, kind="ExternalInput")
    out = nc.dram_tensor("out", [TOK, H], BF, kind="ExternalOutput")

    Exp = mybir.ActivationFunctionType.Exp
    Ln = mybir.ActivationFunctionType.Ln
    Ident = mybir.ActivationFunctionType.Identity

    with tile.TileContext(nc) as tc:
      with (
          tc.tile_pool(name="ctxT", bufs=1) as ctxp,
          tc.tile_pool(name="cc", bufs=1) as ccp,
          tc.tile_pool(name="ln", bufs=10) as lnp,
          tc.tile_pool(name="qT", bufs=NH) as qtp,
          tc.tile_pool(name="kT", bufs=KVH) as ktp,
          tc.tile_pool(name="vv", bufs=2) as vvp,
          tc.tile_pool(name="expp", bufs=36) as expp,
          tc.tile_pool(name="rec", bufs=6) as recp,
          tc.tile_pool(name="bc", bufs=6) as bcp,
          tc.tile_pool(name="const", bufs=1) as constp,
          tc.tile_pool(name="wq", bufs=3) as wqp,
          tc.tile_pool(name="xTp", bufs=1) as xtp,
          tc.tile_pool(name="wo", bufs=1) as wop,
          tc.tile_pool(name="xr", bufs=4) as xrp,
          tc.tile_pool(name="maskp", bufs=SC if use_mask else 1) as mkp,
      ):
        eps_sb = ccp.tile([128, 1], FP, name="eps_sb")
        nb2_sb = ccp.tile([128, 1], FP, name="nb2_sb")
        junk_sb = ccp.tile([128, 1], FP, name="junk_sb")
        wz_sb = ccp.tile([128, 512], F8, name="wz_sb")
        nc.vector.memset(eps_sb[:], EPS)
        nc.vector.memset(nb2_sb[:], -2.0)
        nc.vector.memset(wz_sb[:], 0.0)
        # preload act table set 6 (natural_log_exp_and_others): exp, ln and
        # identity all live there, so no further table switches are needed
        # (ln/exp replace sqrt in phase C's rstd)
        nc.scalar.add_instruction(mybir.InstLoadActFuncSet(
            name=nc.get_next_instruction_name(),
            act_func_set_id=6, ins=[], outs=[],
        ))
        nc.scalar.activation(junk_sb[:], eps_sb[:], func=Exp,
                             bias=eps_sb[:], scale=1.0)
        if not triv_ln:
            gam_sb = ccp.tile([128, H], BF, name="gam_sb")
            bet_sb = ccp.tile([128, H], BF, name="bet_sb")

        ctx_sb = ctxp.tile([128, 16, TOK], F8, name="ctx")
        qT_sb = [qtp.tile([64, TOK], F8, name="qt") for _ in range(NH)]
        kT_sb = [ktp.tile([64, S], F8, name="kt") for _ in range(KVH)]
        # V: [s-part, sc-in-quad, kv-head, 64+1(rowsum)+7(pad to 16B)]
        vv_sb = [vvp.tile([128, 4, KVH, 72], F8, name="vv") for _ in range(2)]
        for t in vv_sb:
            nc.gpsimd.memset(t[:, :, :, 64:65], 1.0)

        _stkA = ExitStack()   # psA: q_proj PSUM, closed after last q_proj
        _stk = ExitStack()
        psA = _stkA.enter_context(
            tc.tile_pool(name="psA", bufs=2, space="PSUM", side="right"))
        psS = _stk.enter_context(tc.tile_pool(name="psS", bufs=2, space="PSUM"))

        id_sb = constp.tile([128, 128], BF, name="id_sb")
        if not triv_bias:
            bias_sb = constp.tile([128, 532], FP, name="bias_sb")
            bq_sb = bias_sb[:, 0:16]
            bk_sb = bias_sb[:, 16:20]
            bv_sb = bias_sb[:, 20:532]

        def q_proj(m):
            blk = wq_chunks[m // 2]
            q = m % 2
            ps = psA.tile([128, TOK], FP, tag="psA", name="psA")
            for c in range(KC):
                nc.tensor.matmul(
                    ps[:],
                    blk[:, q, c],
                    xt_t[:, c],
                    start=(c == 0),
                    stop=(c == KC - 1),
                    perf_mode=DR,
                )
            with nc.allow_low_precision(reason="q cast to fp8 for scores"):
                for hi in range(2):
                    if triv_bias:
                        nc.vector.tensor_copy(
                            qT_sb[2 * m + hi][:],
                            ps[hi * 64:hi * 64 + 64, :],
                        )
                    else:
                        nc.vector.tensor_scalar_add(
                            qT_sb[2 * m + hi][:],
                            ps[hi * 64:hi * 64 + 64, :],
                            bq_sb[hi * 64:hi * 64 + 64, m:m + 1],
                        )

        def scores_quad(hh, qd):
            h = hh // G
            ps = psS.tile([128, 2, TOK], FP, tag="psS", name="psS")
            for i in range(2):
                sc = 2 * qd + i
                # plain fp8 (no DoubleRow): K=64 real rows, no zero pad
                nc.tensor.matmul(
                    ps[:, i, :],
                    kT_sb[h][:, sc * 128:(sc + 1) * 128],
                    qT_sb[hh][:],
                    start=True,
                    stop=True,
                )
                if use_mask:
                    nc.vector.tensor_add(ps[:, i, :], ps[:, i, :],
                                         mask_sb[sc][:])
            ex = expp.tile([128, 2, TOK], F8, tag="ex", name="ex")
            nc.scalar.activation(ex[:], ps[:], func=Exp,
                                 scale=1.0 / 8192.0, bias=nb2_sb[:])
            return ex

        def attn_mms(hh, exs):
            h = hh // G
            po = psO.tile([128, TOK], FP, tag="psO", name="psO")
            for qd in range(4):
                vq = vv_sb[qd // 2][:, (qd % 2) * 2:(qd % 2) * 2 + 2, h, 0:65]
                nc.tensor.matmul(
                    po[0:65, :],
                    vq,
                    exs[qd][:],
                    start=(qd == 0),
                    stop=(qd == 3),
                    perf_mode=DR,
                )
            return po

        def attn_norm(hh, po):
            recb = recp.tile([1, TOK], BF, tag="recb", name="recb")
            with nc.allow_low_precision(reason="softmax recip rounds to bf16"):
                nc.vector.reciprocal(recb[:], po[64:65, :])
            # broadcast recip across 64 partitions on the idle Pool engine
            rb = bcp.tile([64, TOK], BF, tag="rb", name="rb")
            nc.gpsimd.partition_broadcast(rb[:], recb[:])
            with nc.allow_low_precision(reason="ctx cast to fp8 for O-proj"):
                nc.vector.tensor_mul(
                    ctx_sb[(hh % 2) * 64:(hh % 2) * 64 + 64, hh // 2, :],
                    po[0:64, :],
                    rb[:],
                )

        def attn_v(hh, exs):
            attn_norm(hh, attn_mms(hh, exs))

        wq_chunks = {}
        xres_sb = []
        stg_sb = []
        wo_t = wop.tile([128, KC, 2, H], F8, name="wot")

        # ---- Phase A: input DMAs, K/V/Q projections, head-0..7 scores ----
        with (
            tc.tile_pool(name="eTp", bufs=1) as etp,
            tc.tile_pool(name="wk", bufs=1) as wkp,
            tc.tile_pool(name="wv", bufs=1) as wvp,
            tc.tile_pool(name="psKV", bufs=2, space="PSUM") as psKV,
        ):
            wk_t = wkp.tile([128, 4, KC, 2, 128], F8, name="wkt")
            et_t = etp.tile([128, KC, 2, S], F8, name="et")
            wv_t = wvp.tile([128, KC, 2, 512], F8, name="wvt")
            xt_t = xtp.tile([128, KC, 2, TOK], F8, name="xt")
            # PE p-state warm-up during the initial DMA wait
            def warmup(n):
                wps = psA.tile([128, TOK], FP, tag="psA", name="psA")
                for _ in range(n):
                    nc.tensor.matmul(
                        wps[:], wz_sb[:, 0:128], wz_sb[:],
                        start=True, stop=True,
                    )

            warmup(12)

            def wq_chunk(i):
                t = wqp.tile([128, 2, KC, 2, 128], F8, tag="wq", name="wqt")
                nc.sync.dma_start(t[:], wq[:, 2 * i:2 * i + 2])
                wq_chunks[i] = t

            def load_masks():
                for sc in range(SC):
                    mt = mkp.tile([128, TOK], BF, tag="mk", name="mk")
                    nc.sync.dma_start(mt[:], maskT[sc * 128:(sc + 1) * 128, :])
                    mask_sb.append(mt)

            # The DMA-engine pool is an exclusive shared resource: transfers
            # run in acquisition order, so the WHOLE input stream rides the
            # in-order SP queue sorted by first-need time.
            nc.sync.dma_start(et_t[:, :, :, 0:512], eT[:, :, :, 0:512])
            # chunk0 split per m-block: q_proj(0) only needs m0
            t0 = wqp.tile([128, 2, KC, 2, 128], F8, tag="wq", name="wqt")
            nc.sync.dma_start(t0[:, 0:1], wq[:, 0:1])
            wq_chunks[0] = t0
            nc.sync.dma_start(xt_t[:, 0:4], xT[:, 0:4])
            nc.sync.dma_start(wk_t[:, 0], wk[:, 0])
            if not triv_bias:
                nc.sync.dma_start(bias_sb[:], bias_all[:])
            if use_mask:
                mask_sb = []
                load_masks()
            nc.sync.dma_start(xt_t[:, 4:8], xT[:, 4:8])
            nc.sync.dma_start(et_t[:, :, :, 512:S], eT[:, :, :, 512:S])
            nc.sync.dma_start(t0[:, 1:2], wq[:, 1:2])
            wq_chunk(1)
            nc.sync.dma_start(wk_t[:, 1:4], wk[:, 1:4])
            nc.sync.dma_start(wv_t[:], wv[:])
            wq_chunk(2)
            wq_chunk(3)
            nc.sync.dma_start(wo_t[:, :, :, 0:1024], wo[:, :, :, 0:1024])
            nc.sync.dma_start(wo_t[:, :, :, 1024:H], wo[:, :, :, 1024:H])
            wq_chunk(4)

            def xres_dma(tt):
                xt4 = xrp.tile([128, H], BF, tag="xr", name="xr")
                nc.sync.dma_start(xt4[:], xres[tt * 128:(tt + 1) * 128, :])
                xres_sb.append(xt4)

            xres_dma(0)
            wq_chunk(5)
            xres_dma(1)
            wq_chunk(6)
            xres_dma(2)
            wq_chunk(7)
            xres_dma(3)
            nc.sync.dma_start(id_sb[:], ident[:])
            if not triv_ln:
                nc.sync.dma_start(gam_sb[:], gamr[:])
                nc.sync.dma_start(bet_sb[:], betr[:])

            def k_cast(ps, m, sh, hi):
                with nc.allow_low_precision(reason="k cast to fp8"):
                    dst = kT_sb[2 * m + hi][:, sh * 512:(sh + 1) * 512]
                    if triv_bias:
                        nc.vector.tensor_copy(
                            dst, ps[hi * 64:hi * 64 + 64, :])
                    else:
                        nc.vector.tensor_scalar_add(
                            dst,
                            ps[hi * 64:hi * 64 + 64, :],
                            bk_sb[hi * 64:hi * 64 + 64, m:m + 1],
                        )

            def k_proj(m, sh, defer_hi1=False):
                ps = psKV.tile([128, 512], FP, tag="psKV", name="psKV")
                for c in range(KC):
                    nc.tensor.matmul(
                        ps[:],
                        wk_t[:, m, c],
                        et_t[:, c, :, sh * 512:(sh + 1) * 512],
                        start=(c == 0),
                        stop=(c == KC - 1),
                        perf_mode=DR,
                    )
                k_cast(ps, m, sh, 0)
                if not defer_hi1:
                    k_cast(ps, m, sh, 1)
                return ps

            def v_proj(scs):
                pss = [psKV.tile([128, 512], FP, tag="psKV", name="psKV")
                       for _ in scs]
                for c in range(KC):
                    for i, sc in enumerate(scs):
                        nc.tensor.matmul(
                            pss[i][:],
                            et_t[:, c, :, sc * 128:(sc + 1) * 128],
                            wv_t[:, c],
                            start=(c == 0),
                            stop=(c == KC - 1),
                            perf_mode=DR,
                        )
                with nc.allow_low_precision(reason="v cast to fp8"):
                    for i, sc in enumerate(scs):
                        if triv_bias:
                            nc.vector.tensor_copy(
                                vv_sb[sc // 4][:, sc % 4, :, 0:64],
                                pss[i].rearrange("p (h d) -> p h d", d=HD),
                            )
                        else:
                            nc.vector.tensor_add(
                                vv_sb[sc // 4][:, sc % 4, :, 0:64],
                                pss[i].rearrange("p (h d) -> p h d", d=HD),
                                bv_sb.rearrange("p (h d) -> p h d", d=HD),
                            )

            # interleave: get exp started on head 0 ASAP, then keep the
            # stream fed while k/v/q projections drain
            stash = {}

            def sq(hh, *qds):
                for qd in qds:
                    stash[(hh, qd)] = scores_quad(hh, qd)

            psk0 = k_proj(0, 0, defer_hi1=True)  # kT0 s-lo now ...
            q_proj(0)
            q_proj(1)
            sq(0, 0, 1)
            k_cast(psk0, 0, 0, 1)                # ... kT1 s-lo after qT casts
            k_proj(0, 1)                       # kT0,kT1 s-hi
            sq(1, 0)
            sq(0, 2, 3)
            q_proj(2)
            q_proj(3)
            sq(1, 1, 2, 3)
            k_proj(1, 0)                       # kT2,kT3 s-lo
            sq(2, 0, 1, 2, 3)
            k_proj(1, 1)                       # kT2,kT3 s-hi
            sq(3, 0, 1, 2, 3)
            v_proj((0, 1))
            sq(4, 0, 1)
            v_proj((2, 3))
            sq(4, 2, 3)
            sq(5, 0, 1, 2, 3)
            v_proj((4, 5))
            sq(6, 0, 1, 2, 3)
            v_proj((6, 7))
            sq(7, 0, 1, 2, 3)
            k_proj(2, 0)
            k_proj(2, 1)
            k_proj(3, 0)
            k_proj(3, 1)

        # psKV released; open attention output pool in its banks
        psO = _stk.enter_context(tc.tile_pool(name="psO", bufs=2, space="PSUM"))
        # staging SBUF reuses the just-freed phase-A eT/wk/wv space;
        # own stack: it must outlive _stk (phase C reads the staged tiles)
        _stkS = ExitStack()
        stgp = _stkS.enter_context(tc.tile_pool(name="stg", bufs=16))

        psStage = None

        def stage_unit(tt, nb):
            """O-proj chunks 0-5 (heads 0-23) for one (tt,nb) output tile,
            staged to SBUF bf16 in the PSUM banks freed by psA."""
            ps = psStage.tile([128, 512], FP, tag="stg", name="psStg")
            for c in range(6):
                nc.tensor.matmul(
                    ps[:],
                    ctx_sb[:, 2 * c:2 * c + 2, tt * 128:(tt + 1) * 128],
                    wo_t[:, c, :, nb * 512:(nb + 1) * 512],
                    start=(c == 0),
                    stop=(c == 5),
                    perf_mode=DR,
                )
            sg = stgp.tile([128, 512], BF, tag="sg", name="sg")
            with nc.allow_low_precision(reason="O-proj partial staged bf16"):
                nc.vector.tensor_copy(sg[:], ps[:])
            stg_sb.append(sg)

        # ---- Phase B: per-head scores+exp / lagged attn@V ----------------
        pending = []
        norm_q = []
        po31 = None
        for hh in range(NH):
            if hh == 16:
                _stkA.close()   # psA banks -> psStage
                psStage = _stk.enter_context(
                    tc.tile_pool(name="psStage", bufs=2, space="PSUM",
                                 side="right"))
            if hh == 31:
                po31 = psO.tile([128, TOK], FP, tag="psO", name="psO")
            # attn of the lagged head first: its PE mms run while ACT is
            # still chewing the previous head's exps
            lag = 3 if hh < 28 else (2 if hh < 30 else (1 if hh == 30 else 0))
            while len(pending) > lag:
                attn_v(*pending.pop(0))
            exs = []
            for qd in range(4):
                if (hh, qd) in stash:
                    ex = stash.pop((hh, qd))
                else:
                    ex = scores_quad(hh, qd)
                exs.append(ex)
                if hh == 31:
                    # last head: attn mm per quad so only the final quad's
                    # matmul trails the last exp
                    vq = vv_sb[qd // 2][:, (qd % 2) * 2:(qd % 2) * 2 + 2,
                                        hh // G, 0:65]
                    nc.tensor.matmul(
                        po31[0:65, :], vq, ex[:],
                        start=(qd == 0), stop=(qd == 3), perf_mode=DR,
                    )
            if hh < 31:
                pending.append((hh, exs))
            if 4 <= hh <= 15:
                q_proj(hh)      # one per head: smooths the DVE cast load
            # stage O-proj chunks 0-5 once heads 0-23 are all in ctx
            # (attn_v(23) is emitted at hh=26); ~3 units per head
            if hh >= 27:
                u0 = (hh - 27) * 3
                for u in range(u0, min(u0 + 3, 16)):
                    stage_unit(u // 4, u % 4)
        for p in pending:
            attn_v(*p)
        attn_norm(31, po31)
        while len(stg_sb) < 16:
            u = len(stg_sb)
            stage_unit(u // 4, u % 4)

        _stk.close()

        # ---- Phase C: O-proj chunks 6-7 + staged + residual + LN ---------
        with (
            tc.tile_pool(name="psC", bufs=8, space="PSUM") as psC,
            tc.tile_pool(name="outp", bufs=2) as outp,
        ):
            def phase_c_tt(tt):
                ob = outp.tile([128, H], BF, tag="ob", name="ob")
                stats = lnp.tile([128, 4, 6], FP, tag="st", name="st")
                pss = []
                for nb in range(4):
                    ps = psC.tile([128, 512], FP, tag="psC", name="psC")
                    pss.append(ps)
                    sl = slice(nb * 512, (nb + 1) * 512)
                    for c in (6, 7):
                        nc.tensor.matmul(
                            ps[:],
                            ctx_sb[:, 2 * c:2 * c + 2,
                                   tt * 128:(tt + 1) * 128],
                            wo_t[:, c, :, sl],
                            start=(c == 6),
                            stop=False,
                            perf_mode=DR,
                        )
                    # += staged chunks 0-5 (bf16) on the PE
                    nc.tensor.matmul(
                        ps[:],
                        id_sb[:],
                        stg_sb[tt * 4 + nb][:],
                        start=False,
                        stop=False,
                    )
                    # residual add on the PE: += I.T @ xres (bf16)
                    nc.tensor.matmul(
                        ps[:],
                        id_sb[:],
                        xres_sb[tt][:, sl],
                        start=False,
                        stop=True,
                    )
                    nc.vector.bn_stats(stats[:, nb, :], ps[:])
                mv = lnp.tile([128, 2], FP, tag="mv", name="mv")
                nc.vector.bn_aggr(mv[:], stats[:])
                # rstd = exp(-0.5*ln(var+eps)); stays in the exp table set
                lnv = lnp.tile([128, 1], FP, tag="lv", name="lv")
                nc.scalar.activation(
                    lnv[:], mv[:, 1:2], func=Ln, bias=eps_sb[:], scale=1.0
                )
                rstd = lnp.tile([128, 1], FP, tag="rs", name="rs")
                nc.scalar.activation(
                    rstd[:], lnv[:], func=Exp, bias=0.0, scale=-0.5
                )
                # -rstd and -mu*rstd on ACT too: keeps the whole post-aggr
                # LN chain on one engine (no DVE round-trip sem hops)
                rsn = lnp.tile([128, 1], FP, tag="rn", name="rn")
                nc.scalar.activation(
                    rsn[:], rstd[:], func=Ident, bias=0.0, scale=-1.0,
                )
                nmr = lnp.tile([128, 1], FP, tag="nm", name="nm")
                nc.scalar.activation(
                    nmr[:], mv[:, 0:1], func=Ident, bias=0.0, scale=rsn[:],
                )
                for nb in range(4):
                    sl = slice(nb * 512, (nb + 1) * 512)
                    # (ps - mu) * rstd on ACT while DVE streams bn_stats;
                    # the LAST tile splits ACT/DVE (stats are done by then)
                    if tt == 3 and nb % 2 == 0:
                        with nc.allow_low_precision(
                                reason="LN out rounds to bf16"):
                            nc.vector.tensor_scalar(
                                ob[:, sl],
                                pss[nb][:],
                                scalar1=mv[:, 0:1],
                                scalar2=rstd[:],
                                op0=mybir.AluOpType.subtract,
                                op1=mybir.AluOpType.mult,
                            )
                    else:
                        nc.scalar.activation(
                            ob[:, sl], pss[nb][:], func=Ident,
                            bias=nmr[:], scale=rstd[:],
                        )
                    if not triv_ln:
                        eng = nc.vector if nb % 2 == 0 else nc.gpsimd
                        eng.tensor_mul(ob[:, sl], ob[:, sl], gam_sb[:, sl])
                        eng.tensor_add(ob[:, sl], ob[:, sl], bet_sb[:, sl])
                    # stores on SP (idle here); an ACT-queue DMA would stall
                    # the identity stream -- except on the last tile, where
                    # every idle queue issues in parallel to cut the trail
                    if tt == 3:
                        eng = [nc.sync, nc.scalar, nc.gpsimd, nc.scalar][nb]
                    else:
                        eng = nc.sync
                    eng.dma_start(out[tt * 128:(tt + 1) * 128, sl],
                                  ob[:, sl])

            for tt in range(4):
                phase_c_tt(tt)

        _stkS.close()

    nc.compile()
    return nc


def _get_nc(use_mask: bool, triv_ln: bool | None = None,
            triv_bias: bool | None = None):
    if triv_ln is None:
        triv_ln = _LAST_TRIV[0]
    if triv_bias is None:
        triv_bias = _LAST_TRIV[1]
    key = (use_mask, triv_ln, triv_bias)
    if key not in _CACHE:
        _CACHE[key] = _build(use_mask, triv_ln, triv_bias)
    return _CACHE[key]


_LAST_TRIV = [True, True]


def _pack_w_mblk(w, nm):
    """[2048, nm*128] fp32 -> [128, nm, 8, 2, 128] fp8, m-block outer."""
    return np.ascontiguousarray(
        (w * WS).reshape(KC, 2, 128, nm, 128).transpose(2, 3, 0, 1, 4)
    ).astype(FP8)


def _pack_w(w, cols):
    """[2048, cols] fp32 -> [128, 8, 2, cols] fp8 with K-pair layout."""
    return np.ascontiguousarray(
        (w * WS).reshape(KC, 2, 128, cols).transpose(2, 0, 1, 3)
    ).astype(FP8)


def _pack_act(xT, cols):
    """[2048, cols] fp32 (feature-major) -> [128, 8, 2, cols] fp8."""
    return np.ascontiguousarray(
        xT.reshape(KC, 2, 128, cols).transpose(2, 0, 1, 3)
    ).astype(FP8)


def kernel(
    hidden_state,
    encoder_hidden_state,
    encoder_attention_mask,
    Wq, bq, Wk, bk, Wv, bv, Wo, bo, gamma, beta,
):
    hidden_state = np.asarray(hidden_state, dtype=np.float32)
    encoder_hidden_state = np.asarray(encoder_hidden_state, dtype=np.float32)
    encoder_attention_mask = np.asarray(encoder_attention_mask, dtype=np.float32)
    Wq = np.asarray(Wq, dtype=np.float32)
    bq = np.asarray(bq, dtype=np.float32)
    Wk = np.asarray(Wk, dtype=np.float32)
    bk = np.asarray(bk, dtype=np.float32)
    Wv = np.asarray(Wv, dtype=np.float32)
    bv = np.asarray(bv, dtype=np.float32)
    Wo = np.asarray(Wo, dtype=np.float32)
    bo = np.asarray(bo, dtype=np.float32)
    gamma = np.asarray(gamma, dtype=np.float32)
    beta = np.asarray(beta, dtype=np.float32)

    use_mask = bool(np.any(encoder_attention_mask))
    triv_ln = bool(np.all(gamma == 1.0) and np.all(beta == 0.0))
    triv_bias = not (np.any(bq) or np.any(bk) or np.any(bv))
    _LAST_TRIV[0] = triv_ln
    _LAST_TRIV[1] = triv_bias
    nc = _get_nc(use_mask, triv_ln, triv_bias)
    in_maps = _prepare_in_maps(
        hidden_state, encoder_hidden_state, encoder_attention_mask,
        Wq, bq, Wk, bk, Wv, bv, Wo, bo, gamma, beta, use_mask,
    )

    res = run_bass_kernel_spmd(nc, in_maps, core_ids=list(range(8)))
    kernel._last_results = res

    output = np.empty((B, L, H), dtype=np.float32)
    for c in range(8):
        b, lh = c // 2, c % 2
        output[b, lh * TOK:(lh + 1) * TOK, :] = res.results[c]["out"]
    return output


def _prepare_in_maps(
    hidden_state, encoder_hidden_state, encoder_attention_mask,
    Wq, bq, Wk, bk, Wv, bv, Wo, bo, gamma, beta, use_mask,
):
    triv_ln = bool(np.all(gamma == 1.0) and np.all(beta == 0.0))
    triv_bias = not (np.any(bq) or np.any(bk) or np.any(bv))
    wq_p = _pack_w_mblk(Wq, 16)
    wk_p = _pack_w_mblk(Wk, 4)
    wv_p = _pack_w(Wv, KVH * HD)
    wo_p = _pack_w(Wo, H)
    if not triv_bias:
        bias_all = np.ascontiguousarray(np.concatenate(
            [
                bq.reshape(16, 128).T * WS,
                bk.reshape(4, 128).T * WS,
                np.tile(bv[None, :], (128, 1)) * WS,
            ],
            axis=1,
        ).astype(np.float32))
    ident = np.ascontiguousarray(np.eye(128, dtype=np.float32).astype(BF16))
    if not triv_ln:
        gamr = np.ascontiguousarray(
            np.tile(gamma[None, :].astype(BF16), (128, 1)))
        betr = np.ascontiguousarray(
            np.tile(beta[None, :].astype(BF16), (128, 1)))

    eT_by_b = [
        _pack_act(encoder_hidden_state[b].T, S) for b in range(B)
    ]

    in_maps = []
    for c in range(8):
        b, lh = c // 2, c % 2
        rows = hidden_state[b, lh * TOK:(lh + 1) * TOK, :]
        m = {
            "xT": _pack_act(rows.T, TOK),
            "xres": ((rows + bo[None, :]) * (WS * WS)).astype(BF16),
            "eT": eT_by_b[b],
            "wq": wq_p, "wk": wk_p, "wv": wv_p, "wo": wo_p,
            "ident": ident,
        }
        if not triv_bias:
            m["bias_all"] = bias_all
        if not triv_ln:
            m["gamr"] = gamr
            m["betr"] = betr
        if use_mask:
            mslice = encoder_attention_mask[b, 0, lh * TOK:(lh + 1) * TOK, :]
            m["maskT"] = np.ascontiguousarray(
                (mslice.T * (8.0 * WS * WS)).astype(BF16))
        in_maps.append(m)
    return in_maps
